# revision 1
# baseline (speedup 1.0000x reference)
"""Two-layer GAT on 8 Trainium2 NeuronCores (Bass/Tile).

Sharding: edges partitioned by destination node (dst-ownership), 6250 dsts
per core in 49 blocks of 128; weights/attention replicated; the small
inter-layer projection table is AllGathered on device.

Per 128-edge chunk (layer 1): indirect-DMA gather of 256B x rows by src;
PE-transpose + matmul against [W1 | w_as1] -> per-edge h (256) and a_s (4)
in PSUM; a_d[dst] added into the same PSUM slice by a matmul with a
transposed dst-onehot; Lrelu+Exp on ScalarE; softmax denominator and
message aggregation are onehot matmuls PSUM-accumulated over the block;
1/denom applied once per block post-aggregation (max-subtraction skipped:
logits are O(5), f32 exp is safe). Layer 2 gathers 264B rows of the
allgathered [h2 | a_s2 | a_d2] table; same edge pipeline, one head.
"""
import numpy as np

import concourse.bass as bass
import concourse.tile as tile
from concourse import bacc, mybir

N = 50000
E = 800000
F = 64
H = 4
C = 64
HC = H * C
NEG = 0.2
NCORE = 8
NPC = N // NCORE
BLK = 128
NB = (NPC + BLK - 1) // BLK
NPAD = NB * BLK
NFULL = ((N + 127) // 128) * 128
L2W = C + 2  # h2aug row: 64 h2, a_s2, a_d2

f32 = mybir.dt.float32
i32 = mybir.dt.int32

_cache = {}


def _bcast_groups(ap2d, nh, width):
    """[128, nh] AP -> [128, nh, width] with 0-stride inner broadcast."""
    return bass.AP(tensor=ap2d.tensor, offset=ap2d.offset,
                   ap=[ap2d.ap[0], [1, nh], [0, width]])


def build_program(cap, repeat, debug_dump=False, ablate=()):
    # ablate: subset of {"l1","l2","l2prep","cc","l1g","l2g","l1dve","l2dve",
    #                    "l1pe","l2pe","l1act","drep"} to DISABLE.
    nchunk = NB * cap
    nc = bacc.Bacc("TRN2", target_bir_lowering=False, debug=False,
                   num_devices=NCORE)
    AF = mybir.ActivationFunctionType
    EQ = mybir.AluOpType.is_equal

    x_d = nc.dram_tensor("x", [NFULL, F], f32, kind="ExternalInput").ap()
    xto_d = nc.dram_tensor("xt_own", [F, NPAD], f32, kind="ExternalInput").ap()
    esrc_d = nc.dram_tensor("esrc", [128, nchunk], i32, kind="ExternalInput").ap()
    esrc2_d = nc.dram_tensor("esrc2", [128, nchunk], i32, kind="ExternalInput").ap()
    dstl_d = nc.dram_tensor("dstl", [128, nchunk], f32, kind="ExternalInput").ap()
    dstlr_d = nc.dram_tensor("dstlrow", [NB, cap * 128], f32, kind="ExternalInput").ap()
    w1_d = nc.dram_tensor("w1cat", [F, HC + H], f32, kind="ExternalInput").ap()
    wad1_d = nc.dram_tensor("wad1", [F, H], f32, kind="ExternalInput").ap()
    w2_d = nc.dram_tensor("w2cat", [HC, L2W], f32, kind="ExternalInput").ap()
    b1_d = nc.dram_tensor("b1", [1, HC], f32, kind="ExternalInput").ap()
    b2_d = nc.dram_tensor("b2", [1, C], f32, kind="ExternalInput").ap()
    out_d = nc.dram_tensor("out_own", [NPAD, C], f32, kind="ExternalOutput").ap()

    h2o_d = nc.dram_tensor("h2aug_own", [NPAD, L2W], f32)
    h2a_d = nc.dram_tensor("h2aug_all", [NCORE * NPAD, L2W], f32,
                           addr_space="Shared")
    if debug_dump:
        dbg_h1 = nc.dram_tensor("dbg_h1", [NPAD, HC], f32,
                                kind="ExternalOutput").ap()
        dbg_ee = nc.dram_tensor("dbg_ee", [128, NB * cap * H], f32,
                                kind="ExternalOutput").ap()
        dbg_ad = nc.dram_tensor("dbg_ad", [NPAD, H], f32,
                                kind="ExternalOutput").ap()
        dbg_den = nc.dram_tensor("dbg_den", [NPAD, H], f32,
                                 kind="ExternalOutput").ap()
        dbg_h2a = nc.dram_tensor("dbg_h2a", [NCORE * NPAD, L2W], f32,
                                 kind="ExternalOutput").ap()
        dbg_agg = nc.dram_tensor("dbg_agg", [NPAD, HC], f32,
                                 kind="ExternalOutput").ap()
        dbg_msg = nc.dram_tensor("dbg_msg", [128, HC], f32,
                                 kind="ExternalOutput").ap()

    ab = set(ablate)
    with tile.TileContext(nc) as tc:
        with tc.tile_pool(name="one", bufs=1) as one, \
             tc.tile_pool(name="blkp", bufs=2) as blkp, \
             tc.tile_pool(name="chp", bufs=4) as chp, \
             tc.tile_pool(name="ps", bufs=2, space="PSUM") as ps:

            # --- constants ---
            iota_i = one.tile([128, 128], i32)
            nc.gpsimd.iota(iota_i[:], pattern=[[1, 128]], base=0,
                           channel_multiplier=0)
            iota_f = one.tile([128, 128], f32)
            nc.vector.tensor_copy(iota_f[:], iota_i[:])
            iop_i = one.tile([128, 1], i32)
            nc.gpsimd.iota(iop_i[:], pattern=[[0, 1]], base=0,
                           channel_multiplier=1)
            iota_p = one.tile([128, 1], f32)
            nc.vector.tensor_copy(iota_p[:], iop_i[:])
            ident = one.tile([128, 128], f32)
            nc.vector.tensor_scalar(out=ident[:], in0=iota_f[:],
                                    scalar1=iota_p[:, 0:1], scalar2=None,
                                    op0=EQ)

            w1_sb = one.tile([F, HC + H], f32)
            nc.sync.dma_start(out=w1_sb[:], in_=w1_d[:])
            wad1_sb = one.tile([F, H], f32)
            nc.sync.dma_start(out=wad1_sb[:], in_=wad1_d[:])
            w2a_sb = one.tile([128, L2W], f32)
            nc.sync.dma_start(out=w2a_sb[:], in_=w2_d[0:128, :])
            w2b_sb = one.tile([128, L2W], f32)
            nc.sync.dma_start(out=w2b_sb[:], in_=w2_d[128:256, :])
            b1r = one.tile([128, HC], f32)
            nc.sync.dma_start(out=b1r[:], in_=bass.AP(
                tensor=b1_d.tensor, offset=0, ap=[[0, 128], [1, HC]]))
            b2r = one.tile([128, C], f32)
            nc.sync.dma_start(out=b2r[:], in_=bass.AP(
                tensor=b2_d.tensor, offset=0, ap=[[0, 128], [1, C]]))

            esrc_sb = one.tile([128, nchunk], i32)
            nc.sync.dma_start(out=esrc_sb[:], in_=esrc_d[:])
            esrc2_sb = one.tile([128, nchunk], i32)
            nc.sync.dma_start(out=esrc2_sb[:], in_=esrc2_d[:])
            dstl_sb = one.tile([128, nchunk], f32)
            nc.sync.dma_start(out=dstl_sb[:], in_=dstl_d[:])
            xto_sb = one.tile([F, NPAD], f32)
            nc.sync.dma_start(out=xto_sb[:], in_=xto_d[:])

            hout_sb = one.tile([128, NB, HC], f32)
            h2aug_sb = one.tile([128, NB, L2W], f32)

            def load_drep(b):
                drep = blkp.tile([128, cap * 128], f32, tag="drep")
                if "drep" not in ab:
                    nc.sync.dma_start(out=drep[:], in_=bass.AP(
                        tensor=dstlr_d.tensor, offset=b * cap * 128,
                        ap=[[0, 128], [1, cap * 128]]))
                return drep

            for _ in range(repeat):
                # ================= layer 1 =================
                for b in range(NB if "l1" not in ab else 0):
                    ad_ps = ps.tile([128, H], f32, space="PSUM", tag="tr")
                    nc.tensor.matmul(out=ad_ps[:],
                                     lhsT=xto_sb[:, b * 128:(b + 1) * 128],
                                     rhs=wad1_sb[:], start=True, stop=True)
                    adblk = blkp.tile([128, H], f32, tag="adblk")
                    nc.vector.tensor_copy(adblk[:], ad_ps[:])

                    drep = load_drep(b)
                    eeblk = blkp.tile([128, cap, H], f32, tag="eeblk")
                    od_ps = ps.tile([128, HC], f32, space="PSUM", tag="agg")
                    den_ps = ps.tile([128, H], f32, space="PSUM", tag="small")

                    for j in range(cap):
                        cc = b * cap + j
                        xg = chp.tile([128, F], f32, tag="xg")
                        if "l1g" not in ab:
                            nc.gpsimd.indirect_dma_start(
                                out=xg[:], out_offset=None, in_=x_d[:],
                                in_offset=bass.IndirectOffsetOnAxis(
                                    ap=esrc_sb[:, cc:cc + 1], axis=0))
                        xgt_ps = ps.tile([F, 128], f32, space="PSUM", tag="tr")
                        nc.tensor.transpose(out=xgt_ps[:], in_=xg[:],
                                            identity=ident[:])
                        xgt = chp.tile([F, 128], f32, tag="xgts")
                        nc.scalar.copy(xgt[:], xgt_ps[:])

                        he_ps = ps.tile([128, HC + H], f32, space="PSUM",
                                        tag="proj")
                        nc.tensor.matmul(out=he_ps[:], lhsT=xgt[:],
                                         rhs=w1_sb[:], start=True, stop=False,
                                         skip_group_check=True)
                        ohdt = chp.tile([128, 128], f32, tag="ohdt")
                        nc.vector.tensor_scalar(
                            out=ohdt[:], in0=drep[:, j * 128:(j + 1) * 128],
                            scalar1=iota_p[:, 0:1], scalar2=None, op0=EQ)
                        nc.tensor.matmul(out=he_ps[:, HC:HC + H], lhsT=ohdt[:],
                                         rhs=adblk[:], start=False, stop=True,
                                         skip_group_check=True)
                        lg = chp.tile([128, H], f32, tag="lg")
                        nc.scalar.activation(lg[:], he_ps[:, HC:HC + H],
                                             AF.Prelu, alpha=NEG)
                        nc.scalar.activation(eeblk[:, j, :], lg[:], AF.Exp)

                        msg = chp.tile([128, HC], f32, tag="msg")
                        nc.vector.tensor_tensor(
                            out=msg[:].rearrange("p (h c) -> p h c", h=H),
                            in0=he_ps[:, 0:HC].rearrange("p (h c) -> p h c", h=H),
                            in1=_bcast_groups(eeblk[:, j, :], H, C),
                            op=mybir.AluOpType.mult)
                        if debug_dump and b == 0 and j == 0:
                            nc.sync.dma_start(out=dbg_msg[:], in_=msg[:])
                        ohd = chp.tile([128, 128], f32, tag="ohd")
                        nc.vector.tensor_scalar(
                            out=ohd[:], in0=iota_f[:],
                            scalar1=dstl_sb[:, cc:cc + 1], scalar2=None, op0=EQ)
                        nc.tensor.matmul(out=od_ps[:], lhsT=ohd[:],
                                         rhs=msg[:], start=(j == 0),
                                         stop=(j == cap - 1))
                        nc.tensor.matmul(out=den_ps[:], lhsT=ohd[:],
                                         rhs=eeblk[:, j, :], start=(j == 0),
                                         stop=(j == cap - 1))

                    den = blkp.tile([128, H], f32, tag="den")
                    nc.vector.tensor_scalar_add(den[:], den_ps[:],
                                                1e-30)
                    if debug_dump:
                        dag = blkp.tile([128, HC], f32, tag="dag")
                        nc.vector.tensor_copy(dag[:], od_ps[:])
                        nc.sync.dma_start(
                            out=dbg_agg[b * 128:(b + 1) * 128, :], in_=dag[:])
                        nc.sync.dma_start(
                            out=dbg_den[b * 128:(b + 1) * 128, :], in_=den[:])
                        nc.sync.dma_start(
                            out=dbg_ad[b * 128:(b + 1) * 128, :], in_=adblk[:])
                        nc.sync.dma_start(
                            out=dbg_ee[:, b * cap * H:(b + 1) * cap * H],
                            in_=eeblk[:])
                    rden = blkp.tile([128, H], f32, tag="rden")
                    nc.vector.reciprocal(rden[:], den[:])
                    hb = hout_sb[:, b, :]
                    nc.vector.tensor_tensor(
                        out=hb.rearrange("p (h c) -> p h c", h=H),
                        in0=od_ps[:].rearrange("p (h c) -> p h c", h=H),
                        in1=_bcast_groups(rden[:], H, C),
                        op=mybir.AluOpType.mult)
                    nc.vector.tensor_add(hb, hb, b1r[:])
                    nc.vector.tensor_scalar_max(hb, hb, 0.0)
                    if debug_dump:
                        nc.sync.dma_start(
                            out=dbg_h1[b * 128:(b + 1) * 128, :], in_=hb)

                # ===== layer-2 projection (own nodes) + allgather =====
                for b in range(NB if "l2prep" not in ab else 0):
                    hb = hout_sb[:, b, :]
                    h2_ps = ps.tile([128, L2W], f32, space="PSUM", tag="proj")
                    for half in range(2):
                        t_ps = ps.tile([128, 128], f32, space="PSUM", tag="tr")
                        nc.tensor.transpose(
                            out=t_ps[:], in_=hb[:, half * 128:(half + 1) * 128],
                            identity=ident[:])
                        ht = chp.tile([128, 128], f32, tag="ht")
                        if half == 0:
                            nc.scalar.copy(ht[:], t_ps[:])
                        else:
                            nc.vector.tensor_copy(ht[:], t_ps[:])
                        nc.tensor.matmul(
                            out=h2_ps[:], lhsT=ht[:],
                            rhs=(w2a_sb[:] if half == 0 else w2b_sb[:]),
                            start=(half == 0), stop=(half == 1))
                    nc.vector.tensor_copy(h2aug_sb[:, b, :], h2_ps[:])
                    nc.sync.dma_start(out=h2o_d[b * 128:(b + 1) * 128, :],
                                      in_=h2aug_sb[:, b, :])

                if "cc" not in ab:
                    nc.gpsimd.collective_compute(
                        "AllGather", mybir.AluOpType.bypass,
                        replica_groups=[list(range(NCORE))],
                        ins=[h2o_d[:]], outs=[h2a_d[:]])

                if debug_dump:
                    dcp = blkp.tile([128, L2W], f32, tag="dcp")
                    for bb in range(NCORE * NPAD // 128):
                        nc.sync.dma_start(
                            out=dcp[:], in_=h2a_d[bb * 128:(bb + 1) * 128, :])
                        nc.sync.dma_start(
                            out=dbg_h2a[bb * 128:(bb + 1) * 128, :], in_=dcp[:])
                # ================= layer 2 =================
                for b in range(NB if "l2" not in ab else 0):
                    drep = load_drep(b)
                    g2blk = blkp.tile([128, cap, L2W], f32, tag="g2blk")
                    lg2 = blkp.tile([128, cap], f32, tag="lg2")
                    for j in range(cap):
                        cc = b * cap + j
                        if "l2g" not in ab:
                            nc.gpsimd.indirect_dma_start(
                                out=g2blk[:, j, :], out_offset=None, in_=h2a_d[:],
                                in_offset=bass.IndirectOffsetOnAxis(
                                    ap=esrc2_sb[:, cc:cc + 1], axis=0))
                        ohdt = chp.tile([128, 128], f32, tag="ohdt")
                        nc.vector.tensor_scalar(
                            out=ohdt[:], in0=drep[:, j * 128:(j + 1) * 128],
                            scalar1=iota_p[:, 0:1], scalar2=None, op0=EQ)
                        ad2_ps = ps.tile([128, 1], f32, space="PSUM",
                                         tag="tr")
                        nc.tensor.matmul(out=ad2_ps[:], lhsT=ohdt[:],
                                         rhs=h2aug_sb[:, b, L2W - 1:L2W],
                                         start=True, stop=True)
                        nc.vector.tensor_tensor(
                            out=lg2[:, j:j + 1], in0=g2blk[:, j, C:C + 1],
                            in1=ad2_ps[:], op=mybir.AluOpType.add)
                    lr2 = blkp.tile([128, cap], f32, tag="lr2")
                    nc.scalar.activation(lr2[:], lg2[:], AF.Prelu, alpha=NEG)
                    ee2 = blkp.tile([128, cap], f32, tag="ee2")
                    nc.scalar.activation(ee2[:], lr2[:], AF.Exp)

                    od2_ps = ps.tile([128, C], f32, space="PSUM", tag="agg")
                    den2_ps = ps.tile([128, 1], f32, space="PSUM", tag="small")
                    for j in range(cap):
                        cc = b * cap + j
                        msg2 = chp.tile([128, C], f32, tag="msg2")
                        nc.vector.tensor_scalar_mul(
                            msg2[:], g2blk[:, j, 0:C], ee2[:, j:j + 1])
                        ohd = chp.tile([128, 128], f32, tag="ohd")
                        nc.vector.tensor_scalar(
                            out=ohd[:], in0=iota_f[:],
                            scalar1=dstl_sb[:, cc:cc + 1], scalar2=None, op0=EQ)
                        nc.tensor.matmul(out=od2_ps[:], lhsT=ohd[:],
                                         rhs=msg2[:], start=(j == 0),
                                         stop=(j == cap - 1))
                        nc.tensor.matmul(out=den2_ps[:], lhsT=ohd[:],
                                         rhs=ee2[:, j:j + 1], start=(j == 0),
                                         stop=(j == cap - 1))

                    den2 = blkp.tile([128, 1], f32, tag="den2")
                    nc.vector.tensor_scalar_add(den2[:], den2_ps[:],
                                                1e-30)
                    rd2 = blkp.tile([128, 1], f32, tag="rd2")
                    nc.vector.reciprocal(rd2[:], den2[:])
                    o2 = blkp.tile([128, C], f32, tag="o2")
                    nc.vector.tensor_scalar_mul(o2[:], od2_ps[:],
                                                rd2[:, 0:1])
                    nc.vector.tensor_add(o2[:], o2[:], b2r[:])
                    nc.sync.dma_start(out=out_d[b * 128:(b + 1) * 128, :],
                                      in_=o2[:])

    nc.compile()
    return nc


def preprocess(x, edge_index, W1, att_src1, att_dst1, b1, W2, att_src2,
               att_dst2, b2):
    x = np.asarray(x, dtype=np.float32)
    src = np.asarray(edge_index[0], dtype=np.int64)
    dst = np.asarray(edge_index[1], dtype=np.int64)
    W1 = np.asarray(W1, dtype=np.float32)
    W2 = np.asarray(W2, dtype=np.float32)

    W1r = W1.reshape(F, H, C)
    w_as1 = np.einsum("khc,hc->kh", W1r, np.asarray(att_src1, np.float32))
    w_ad1 = np.einsum("khc,hc->kh", W1r, np.asarray(att_dst1, np.float32))
    w1cat = np.ascontiguousarray(np.concatenate([W1, w_as1], axis=1))
    w_as2 = W2 @ np.asarray(att_src2, np.float32)[0]
    w_ad2 = W2 @ np.asarray(att_dst2, np.float32)[0]
    w2cat = np.ascontiguousarray(
        np.concatenate([W2, w_as2[:, None], w_ad2[:, None]], axis=1))

    x_pad = np.zeros((NFULL, F), np.float32)
    x_pad[:N] = x
    xt = np.ascontiguousarray(x_pad.T)

    core_of = dst // NPC
    counts = np.zeros((NCORE, NB), np.int64)
    edata = []
    for c in range(NCORE):
        m = core_of == c
        s_c = src[m].astype(np.int32)
        d_c = (dst[m] - c * NPC).astype(np.int32)
        b_c = d_c // BLK
        order = np.argsort(b_c, kind="stable")
        s_c, d_c, b_c = s_c[order], d_c[order], b_c[order]
        cnt = np.bincount(b_c, minlength=NB)
        counts[c] = cnt
        edata.append((s_c, d_c, b_c, cnt))
    cap = int(np.ceil(counts.max() / BLK))
    nchunk = NB * cap

    per_core = []
    for c in range(NCORE):
        s_c, d_c, b_c, cnt = edata[c]
        esrc = np.zeros((128, nchunk), np.int32)
        dstl = np.full((128, nchunk), 128.0, np.float32)
        offs = np.concatenate([[0], np.cumsum(cnt)[:-1]])
        pos = np.arange(len(s_c)) - offs[b_c]
        col = b_c * cap + pos // 128
        row = pos % 128
        esrc[row, col] = s_c
        dstl[row, col] = (d_c - b_c * BLK).astype(np.float32)
        dstlrow = np.full((NB, cap * 128), 128.0, np.float32)
        dstlrow[b_c, pos] = (d_c - b_c * BLK).astype(np.float32)
        esrc2 = (esrc // NPC) * NPAD + esrc % NPC
        per_core.append({
            "x": x_pad, "esrc2": esrc2.astype(np.int32),
            "xt_own": np.ascontiguousarray(xt[:, c * NPC:c * NPC + NPAD]),
            "esrc": esrc, "dstl": dstl, "dstlrow": dstlrow,
            "w1cat": w1cat, "wad1": w_ad1, "w2cat": w2cat,
            "b1": np.asarray(b1, np.float32).reshape(1, HC),
            "b2": np.asarray(b2, np.float32).reshape(1, C),
        })
    return per_core, cap


# ---------- cached PJRT runner (axon path), self-contained ----------

def make_runner(nc, n_cores):
    import jax
    from jax.sharding import Mesh, PartitionSpec, NamedSharding
    from jax.experimental.shard_map import shard_map
    from concourse import mybir as mb
    from concourse.bass2jax import (_bass_exec_p, install_neuronx_cc_hook,
                                    partition_id_tensor)

    install_neuronx_cc_hook()
    in_names, out_names, out_avals = [], [], []
    pid_name = nc.partition_id_tensor.name if nc.partition_id_tensor else None
    for alloc in nc.m.functions[0].allocations:
        if not isinstance(alloc, mb.MemoryLocationSet):
            continue
        name = alloc.memorylocations[0].name
        if alloc.kind == "ExternalInput":
            if name != pid_name:
                in_names.append(name)
        elif alloc.kind == "ExternalOutput":
            out_names.append(name)
            out_avals.append(jax.core.ShapedArray(
                tuple(alloc.tensor_shape), mb.dt.np(alloc.dtype)))
    n_params = len(in_names)
    all_in = in_names + out_names + ([pid_name] if pid_name else [])
    donate = tuple(range(n_params, n_params + len(out_names)))

    def _body(*args):
        args = list(args)
        if pid_name is not None:
            args.append(partition_id_tensor())
        return tuple(_bass_exec_p.bind(
            *args, out_avals=tuple(out_avals), in_names=tuple(all_in),
            out_names=tuple(out_names), lowering_input_output_aliases=(),
            sim_require_finite=True, sim_require_nnan=True, nc=nc))

    devices = jax.devices()[:n_cores]
    mesh = Mesh(np.asarray(devices), ("core",))
    jit_fn = jax.jit(
        shard_map(_body, mesh=mesh,
                  in_specs=(PartitionSpec("core"),) * (n_params + len(out_names)),
                  out_specs=(PartitionSpec("core"),) * len(out_names)),
        donate_argnums=donate, keep_unused=True)
    sharding = NamedSharding(mesh, PartitionSpec("core"))
    dev_cache = {}

    def run(in_maps, time_only=False):
        import time as _t
        ins = []
        for name in in_names:
            key = (name,) + tuple(id(m[name]) for m in in_maps)
            if key not in dev_cache:
                cat = np.concatenate([np.asarray(m[name]) for m in in_maps],
                                     axis=0)
                dev_cache[key] = jax.device_put(cat, sharding)
            ins.append(dev_cache[key])
        zeros = [jax.device_put(
            np.zeros((av.shape[0] * n_cores,) + av.shape[1:], av.dtype),
            sharding) for av in out_avals]
        jax.block_until_ready(zeros)
        t0 = _t.time()
        r = jit_fn(*ins, *zeros)
        jax.block_until_ready(r)
        dt = _t.time() - t0
        if time_only:
            return dt
        outs = []
        for ci in range(n_cores):
            d = {}
            for i, nm in enumerate(out_names):
                full = np.asarray(r[i])
                per = full.shape[0] // n_cores
                d[nm] = full[ci * per:(ci + 1) * per]
            outs.append(d)
        return outs, dt

    return run


def get_runner(cap, repeat=1):
    key = (cap, repeat)
    if key not in _cache:
        nc = build_program(cap, repeat)
        _cache[key] = make_runner(nc, NCORE)
    return _cache[key]


def kernel(**inputs):
    per_core, cap = preprocess(**inputs)
    run = get_runner(cap, 1)
    res, _ = run(per_core)
    out = np.concatenate([res[c]["out_own"][:NPC] for c in range(NCORE)],
                         axis=0)
    return np.ascontiguousarray(out, dtype=np.float32)



# revision 5
# speedup vs baseline: 2.3094x; 2.3094x over previous
"""Two-layer GAT on 8 Trainium2 NeuronCores (Bass/Tile).

Sharding: edges partitioned by destination node (dst-ownership), 6250 dsts
per core in 49 blocks of 128; weights/attention replicated; the small
inter-layer projection table is AllGathered on device.

Per 128-edge chunk (layer 1): indirect-DMA gather of 256B x rows by src;
PE-transpose + matmul against [W1 | w_as1] -> per-edge h (256) and a_s (4)
in PSUM; a_d[dst] added into the same PSUM slice by a matmul with a
transposed dst-onehot; Lrelu+Exp on ScalarE; softmax denominator and
message aggregation are onehot matmuls PSUM-accumulated over the block;
1/denom applied once per block post-aggregation (max-subtraction skipped:
logits are O(5), f32 exp is safe). Layer 2 gathers 264B rows of the
allgathered [h2 | a_s2 | a_d2] table; same edge pipeline, one head.
"""
import numpy as np

import concourse.bass as bass
import concourse.tile as tile
from concourse import bacc, mybir

N = 50000
E = 800000
F = 64
H = 4
C = 64
HC = H * C
NEG = 0.2
NCORE = 8
NPC = N // NCORE
BLK = 128
NB = (NPC + BLK - 1) // BLK
NPAD = NB * BLK
NFULL = ((N + 127) // 128) * 128
L2W = C + 2  # h2aug row: 64 h2, a_s2, a_d2
# partial-allgather issue points: after block b, gather rows [r0, r1)
CC_AT = {12: (0, 1664), 25: (1664, 3328), 38: (3328, 4992),
         48: (4992, NPAD)}

f32 = mybir.dt.float32
f32r = mybir.dt.float32r
i32 = mybir.dt.int32

_cache = {}


def _bcast_groups(ap2d, nh, width):
    """[128, nh] AP -> [128, nh, width] with 0-stride inner broadcast."""
    return bass.AP(tensor=ap2d.tensor, offset=ap2d.offset,
                   ap=[ap2d.ap[0], [1, nh], [0, width]])


def build_program(cap, repeat, debug_dump=False, ablate=()):
    # ablate: subset of {"l1","l2","l2prep","cc","l1g","l2g","l1dve","l2dve",
    #                    "l1pe","l2pe","l1act","drep"} to DISABLE.
    nchunk = NB * cap
    nc = bacc.Bacc("TRN2", target_bir_lowering=False, debug=False,
                   num_devices=NCORE)
    AF = mybir.ActivationFunctionType
    EQ = mybir.AluOpType.is_equal

    x_d = nc.dram_tensor("x", [NFULL, F], f32, kind="ExternalInput").ap()
    xto_d = nc.dram_tensor("xt_own", [F, NPAD], f32, kind="ExternalInput").ap()
    esrc_d = nc.dram_tensor("esrc", [128, nchunk], i32, kind="ExternalInput").ap()
    esrc2_d = nc.dram_tensor("esrc2", [128, nchunk], i32, kind="ExternalInput").ap()
    dstl_d = nc.dram_tensor("dstl", [128, nchunk], f32, kind="ExternalInput").ap()
    dstlr_d = nc.dram_tensor("dstlrow", [NB, cap * 128], f32, kind="ExternalInput").ap()
    w1_d = nc.dram_tensor("w1cat", [F, HC + H], f32, kind="ExternalInput").ap()
    wad1_d = nc.dram_tensor("wad1", [F, H], f32, kind="ExternalInput").ap()
    w2_d = nc.dram_tensor("w2cat", [HC, L2W], f32, kind="ExternalInput").ap()
    b1_d = nc.dram_tensor("b1", [1, HC], f32, kind="ExternalInput").ap()
    b2_d = nc.dram_tensor("b2", [1, C], f32, kind="ExternalInput").ap()
    out_d = nc.dram_tensor("out_own", [NPAD, C], f32, kind="ExternalOutput").ap()

    h2o_d = nc.dram_tensor("h2aug_own", [NPAD, L2W], f32)
    h2a_d = nc.dram_tensor("h2aug_all", [NCORE * NPAD, L2W], f32,
                           addr_space="Shared")
    if debug_dump:
        dbg_h1 = nc.dram_tensor("dbg_h1", [NPAD, HC], f32,
                                kind="ExternalOutput").ap()
        dbg_ee = nc.dram_tensor("dbg_ee", [128, NB * cap * H], f32,
                                kind="ExternalOutput").ap()
        dbg_ad = nc.dram_tensor("dbg_ad", [NPAD, H], f32,
                                kind="ExternalOutput").ap()
        dbg_den = nc.dram_tensor("dbg_den", [NPAD, H], f32,
                                 kind="ExternalOutput").ap()
        dbg_h2a = nc.dram_tensor("dbg_h2a", [NCORE * NPAD, L2W], f32,
                                 kind="ExternalOutput").ap()
        dbg_agg = nc.dram_tensor("dbg_agg", [NPAD, HC], f32,
                                 kind="ExternalOutput").ap()
        dbg_msg = nc.dram_tensor("dbg_msg", [128, HC], f32,
                                 kind="ExternalOutput").ap()

    ab = set(ablate)
    with tile.TileContext(nc) as tc:
        with tc.tile_pool(name="one", bufs=1) as one, \
             tc.tile_pool(name="blkp", bufs=2) as blkp, \
             tc.tile_pool(name="chp", bufs=4) as chp, \
             tc.tile_pool(name="ps", bufs=2, space="PSUM") as ps:

            # --- constants ---
            iota_i = one.tile([128, 128], i32)
            nc.gpsimd.iota(iota_i[:], pattern=[[1, 128]], base=0,
                           channel_multiplier=0)
            iota_f = one.tile([128, 128], f32)
            nc.vector.tensor_copy(iota_f[:], iota_i[:])
            iop_i = one.tile([128, 1], i32)
            nc.gpsimd.iota(iop_i[:], pattern=[[0, 1]], base=0,
                           channel_multiplier=1)
            iota_p = one.tile([128, 1], f32)
            nc.vector.tensor_copy(iota_p[:], iop_i[:])
            ident = one.tile([128, 128], f32)
            nc.vector.tensor_scalar(out=ident[:], in0=iota_f[:],
                                    scalar1=iota_p[:, 0:1], scalar2=None,
                                    op0=EQ)

            w1_f = one.tile([F, HC + H], f32)
            nc.sync.dma_start(out=w1_f[:], in_=w1_d[:])
            w1_sb = one.tile([F, HC + H], f32r)
            nc.vector.tensor_copy(w1_sb[:], w1_f[:])
            wad1_sb = one.tile([F, H], f32)
            nc.sync.dma_start(out=wad1_sb[:], in_=wad1_d[:])
            w2a_f = one.tile([128, L2W], f32)
            nc.sync.dma_start(out=w2a_f[:], in_=w2_d[0:128, :])
            w2a_sb = one.tile([128, L2W], f32r)
            nc.vector.tensor_copy(w2a_sb[:], w2a_f[:])
            w2b_f = one.tile([128, L2W], f32)
            nc.sync.dma_start(out=w2b_f[:], in_=w2_d[128:256, :])
            w2b_sb = one.tile([128, L2W], f32r)
            nc.vector.tensor_copy(w2b_sb[:], w2b_f[:])
            b1r = one.tile([128, HC], f32)
            nc.sync.dma_start(out=b1r[:], in_=bass.AP(
                tensor=b1_d.tensor, offset=0, ap=[[0, 128], [1, HC]]))
            b2r = one.tile([128, C], f32)
            nc.sync.dma_start(out=b2r[:], in_=bass.AP(
                tensor=b2_d.tensor, offset=0, ap=[[0, 128], [1, C]]))

            esrc_sb = one.tile([128, nchunk], i32)
            nc.sync.dma_start(out=esrc_sb[:], in_=esrc_d[:])
            esrc2_sb = one.tile([128, nchunk], i32)
            nc.sync.dma_start(out=esrc2_sb[:], in_=esrc2_d[:])
            dstl_sb = one.tile([128, nchunk], f32)
            nc.sync.dma_start(out=dstl_sb[:], in_=dstl_d[:])
            xto_sb = one.tile([F, NPAD], f32)
            nc.sync.dma_start(out=xto_sb[:], in_=xto_d[:])

            h2aug_sb = one.tile([128, NB, L2W], f32)

            def load_drep(b):
                drep = blkp.tile([128, cap * 128], f32, tag="drep")
                if "drep" not in ab:
                    nc.sync.dma_start(out=drep[:], in_=bass.AP(
                        tensor=dstlr_d.tensor, offset=b * cap * 128,
                        ap=[[0, 128], [1, cap * 128]]))
                return drep

            for _ in range(repeat):
                # ================= layer 1 =================
                for b in range(NB if "l1" not in ab else 0):
                    ad_ps = ps.tile([128, H], f32, space="PSUM", tag="tr")
                    nc.tensor.matmul(out=ad_ps[:],
                                     lhsT=xto_sb[:, b * 128:(b + 1) * 128],
                                     rhs=wad1_sb[:], start=True, stop=True)
                    adblk = blkp.tile([128, H], f32, tag="adblk")
                    nc.vector.tensor_copy(adblk[:], ad_ps[:])

                    drep = load_drep(b)
                    od_ps = ps.tile([128, HC + H], f32, space="PSUM", tag="agg")

                    for j in range(cap):
                        cc = b * cap + j
                        xg = chp.tile([128, F], f32, tag="xg")
                        if "l1g" not in ab:
                            nc.gpsimd.indirect_dma_start(
                                out=xg[:], out_offset=None, in_=x_d[:],
                                in_offset=bass.IndirectOffsetOnAxis(
                                    ap=esrc_sb[:, cc:cc + 1], axis=0))
                        xgt_ps = ps.tile([F, 128], f32, space="PSUM", tag="tr")
                        nc.tensor.transpose(out=xgt_ps[:], in_=xg[:],
                                            identity=ident[:])
                        xgt = chp.tile([F, 128], f32r, tag="xgts")
                        nc.scalar.copy(xgt[:], xgt_ps[:])

                        he_ps = ps.tile([128, HC + H], f32, space="PSUM",
                                        tag="proj")
                        nc.tensor.matmul(out=he_ps[:], lhsT=xgt[:],
                                         rhs=w1_sb[:], start=True, stop=False,
                                         skip_group_check=True)
                        ohdt = chp.tile([128, 128], f32, tag="ohdt")
                        nc.vector.tensor_scalar(
                            out=ohdt[:], in0=drep[:, j * 128:(j + 1) * 128],
                            scalar1=iota_p[:, 0:1], scalar2=None, op0=EQ)
                        nc.tensor.matmul(out=he_ps[:, HC:HC + H], lhsT=ohdt[:],
                                         rhs=adblk[:], start=False, stop=True,
                                         skip_group_check=True)
                        lg = chp.tile([128, H], f32, tag="lg")
                        nc.scalar.activation(lg[:], he_ps[:, HC:HC + H],
                                             AF.Prelu, alpha=NEG)
                        msg = chp.tile([128, HC + H], f32r, tag="msg")
                        nc.scalar.activation(msg[:, HC:HC + H], lg[:], AF.Exp)
                        nc.vector.tensor_tensor(
                            out=msg[:, 0:HC].rearrange("p (h c) -> p h c", h=H),
                            in0=he_ps[:, 0:HC].rearrange("p (h c) -> p h c", h=H),
                            in1=_bcast_groups(msg[:, HC:HC + H], H, C),
                            op=mybir.AluOpType.mult)
                        ohd = chp.tile([128, 128], f32r, tag="ohd")
                        nc.vector.tensor_scalar(
                            out=ohd[:], in0=iota_f[:],
                            scalar1=dstl_sb[:, cc:cc + 1], scalar2=None, op0=EQ)
                        nc.tensor.matmul(out=od_ps[:], lhsT=ohd[:],
                                         rhs=msg[:], start=(j == 0),
                                         stop=(j == cap - 1))

                    den = blkp.tile([128, H], f32, tag="den")
                    nc.vector.tensor_scalar_add(den[:], od_ps[:, HC:HC + H],
                                                1e-30)
                    if debug_dump:
                        dag = blkp.tile([128, HC], f32, tag="dag")
                        nc.vector.tensor_copy(dag[:], od_ps[:])
                        nc.sync.dma_start(
                            out=dbg_agg[b * 128:(b + 1) * 128, :], in_=dag[:])
                        nc.sync.dma_start(
                            out=dbg_den[b * 128:(b + 1) * 128, :], in_=den[:])
                        nc.sync.dma_start(
                            out=dbg_ad[b * 128:(b + 1) * 128, :], in_=adblk[:])
                        nc.sync.dma_start(
                            out=dbg_ee[:, b * cap * H:(b + 1) * cap * H],
                            in_=eeblk[:])
                    rden = blkp.tile([128, H], f32, tag="rden")
                    nc.vector.reciprocal(rden[:], den[:])
                    hbt = blkp.tile([128, HC], f32, tag="hb")
                    hb = hbt[:]
                    nc.vector.tensor_tensor(
                        out=hb.rearrange("p (h c) -> p h c", h=H),
                        in0=od_ps[:, 0:HC].rearrange("p (h c) -> p h c", h=H),
                        in1=_bcast_groups(rden[:], H, C),
                        op=mybir.AluOpType.mult)
                    nc.vector.tensor_add(hb, hb, b1r[:])
                    nc.vector.tensor_scalar_max(hb, hb, 0.0)
                    if debug_dump:
                        nc.sync.dma_start(
                            out=dbg_h1[b * 128:(b + 1) * 128, :], in_=hb)

                    # fused layer-2 projection for this block
                    h2_ps = ps.tile([128, L2W], f32, space="PSUM", tag="proj")
                    for half in range(2):
                        t_ps = ps.tile([128, 128], f32, space="PSUM", tag="tr")
                        nc.tensor.transpose(
                            out=t_ps[:], in_=hb[:, half * 128:(half + 1) * 128],
                            identity=ident[:])
                        ht = chp.tile([128, 128], f32r, tag="ht")
                        if half == 0:
                            nc.scalar.copy(ht[:], t_ps[:])
                        else:
                            nc.vector.tensor_copy(ht[:], t_ps[:])
                        nc.tensor.matmul(
                            out=h2_ps[:], lhsT=ht[:],
                            rhs=(w2a_sb[:] if half == 0 else w2b_sb[:]),
                            start=(half == 0), stop=(half == 1))
                    nc.vector.tensor_copy(h2aug_sb[:, b, :], h2_ps[:])
                    nc.sync.dma_start(out=h2o_d[b * 128:(b + 1) * 128, :],
                                      in_=h2aug_sb[:, b, :])

                    if "cc" not in ab and b in CC_AT:
                        r0, r1 = CC_AT[b]
                        nc.gpsimd.collective_compute(
                            "AllGather", mybir.AluOpType.bypass,
                            replica_groups=[list(range(NCORE))],
                            ins=[h2o_d[r0:r1, :]],
                            outs=[h2a_d[NCORE * r0:NCORE * r1, :]])

                if debug_dump:
                    dcp = blkp.tile([128, L2W], f32, tag="dcp")
                    for bb in range(NCORE * NPAD // 128):
                        nc.sync.dma_start(
                            out=dcp[:], in_=h2a_d[bb * 128:(bb + 1) * 128, :])
                        nc.sync.dma_start(
                            out=dbg_h2a[bb * 128:(bb + 1) * 128, :], in_=dcp[:])
                # ================= layer 2 =================
                for b in range(NB if "l2" not in ab else 0):
                    drep = load_drep(b)
                    g2blk = blkp.tile([128, cap, L2W], f32, tag="g2blk")
                    lg2 = blkp.tile([128, cap], f32, tag="lg2")
                    for j in range(cap):
                        cc = b * cap + j
                        if "l2g" not in ab:
                            nc.gpsimd.indirect_dma_start(
                                out=g2blk[:, j, :], out_offset=None, in_=h2a_d[:],
                                in_offset=bass.IndirectOffsetOnAxis(
                                    ap=esrc2_sb[:, cc:cc + 1], axis=0))
                        ohdt = chp.tile([128, 128], f32, tag="ohdt")
                        nc.vector.tensor_scalar(
                            out=ohdt[:], in0=drep[:, j * 128:(j + 1) * 128],
                            scalar1=iota_p[:, 0:1], scalar2=None, op0=EQ)
                        ad2_ps = ps.tile([128, 1], f32, space="PSUM",
                                         tag="tr")
                        nc.tensor.matmul(out=ad2_ps[:], lhsT=ohdt[:],
                                         rhs=h2aug_sb[:, b, L2W - 1:L2W],
                                         start=True, stop=True)
                        nc.vector.tensor_tensor(
                            out=lg2[:, j:j + 1], in0=g2blk[:, j, C:C + 1],
                            in1=ad2_ps[:], op=mybir.AluOpType.add)
                    lr2 = blkp.tile([128, cap], f32, tag="lr2")
                    nc.scalar.activation(lr2[:], lg2[:], AF.Prelu, alpha=NEG)
                    ee2 = blkp.tile([128, cap], f32, tag="ee2")
                    nc.scalar.activation(ee2[:], lr2[:], AF.Exp)

                    od2_ps = ps.tile([128, C], f32, space="PSUM", tag="agg")
                    den2_ps = ps.tile([128, 1], f32, space="PSUM", tag="small")
                    for j in range(cap):
                        cc = b * cap + j
                        msg2 = chp.tile([128, C], f32, tag="msg2")
                        nc.vector.tensor_scalar_mul(
                            msg2[:], g2blk[:, j, 0:C], ee2[:, j:j + 1])
                        ohd = chp.tile([128, 128], f32, tag="ohd")
                        nc.vector.tensor_scalar(
                            out=ohd[:], in0=iota_f[:],
                            scalar1=dstl_sb[:, cc:cc + 1], scalar2=None, op0=EQ)
                        nc.tensor.matmul(out=od2_ps[:], lhsT=ohd[:],
                                         rhs=msg2[:], start=(j == 0),
                                         stop=(j == cap - 1))
                        nc.tensor.matmul(out=den2_ps[:], lhsT=ohd[:],
                                         rhs=ee2[:, j:j + 1], start=(j == 0),
                                         stop=(j == cap - 1))

                    den2 = blkp.tile([128, 1], f32, tag="den2")
                    nc.vector.tensor_scalar_add(den2[:], den2_ps[:],
                                                1e-30)
                    rd2 = blkp.tile([128, 1], f32, tag="rd2")
                    nc.vector.reciprocal(rd2[:], den2[:])
                    o2 = blkp.tile([128, C], f32, tag="o2")
                    nc.vector.tensor_scalar_mul(o2[:], od2_ps[:],
                                                rd2[:, 0:1])
                    nc.vector.tensor_add(o2[:], o2[:], b2r[:])
                    nc.sync.dma_start(out=out_d[b * 128:(b + 1) * 128, :],
                                      in_=o2[:])

    nc.compile()
    return nc


def preprocess(x, edge_index, W1, att_src1, att_dst1, b1, W2, att_src2,
               att_dst2, b2):
    x = np.asarray(x, dtype=np.float32)
    src = np.asarray(edge_index[0], dtype=np.int64)
    dst = np.asarray(edge_index[1], dtype=np.int64)
    W1 = np.asarray(W1, dtype=np.float32)
    W2 = np.asarray(W2, dtype=np.float32)

    W1r = W1.reshape(F, H, C)
    w_as1 = np.einsum("khc,hc->kh", W1r, np.asarray(att_src1, np.float32))
    w_ad1 = np.einsum("khc,hc->kh", W1r, np.asarray(att_dst1, np.float32))
    w1cat = np.ascontiguousarray(np.concatenate([W1, w_as1], axis=1))
    w_as2 = W2 @ np.asarray(att_src2, np.float32)[0]
    w_ad2 = W2 @ np.asarray(att_dst2, np.float32)[0]
    w2cat = np.ascontiguousarray(
        np.concatenate([W2, w_as2[:, None], w_ad2[:, None]], axis=1))

    x_pad = np.zeros((NFULL, F), np.float32)
    x_pad[:N] = x
    xt = np.ascontiguousarray(x_pad.T)

    core_of = dst // NPC
    counts = np.zeros((NCORE, NB), np.int64)
    edata = []
    for c in range(NCORE):
        m = core_of == c
        s_c = src[m].astype(np.int32)
        d_c = (dst[m] - c * NPC).astype(np.int32)
        b_c = d_c // BLK
        order = np.argsort(b_c, kind="stable")
        s_c, d_c, b_c = s_c[order], d_c[order], b_c[order]
        cnt = np.bincount(b_c, minlength=NB)
        counts[c] = cnt
        edata.append((s_c, d_c, b_c, cnt))
    cap = int(np.ceil(counts.max() / BLK))
    nchunk = NB * cap

    per_core = []
    for c in range(NCORE):
        s_c, d_c, b_c, cnt = edata[c]
        esrc = np.zeros((128, nchunk), np.int32)
        dstl = np.full((128, nchunk), 128.0, np.float32)
        offs = np.concatenate([[0], np.cumsum(cnt)[:-1]])
        pos = np.arange(len(s_c)) - offs[b_c]
        col = b_c * cap + pos // 128
        row = pos % 128
        esrc[row, col] = s_c
        dstl[row, col] = (d_c - b_c * BLK).astype(np.float32)
        dstlrow = np.full((NB, cap * 128), 128.0, np.float32)
        dstlrow[b_c, pos] = (d_c - b_c * BLK).astype(np.float32)
        cc_bounds = [0, 1664, 3328, 4992, NPAD]
        e_c = esrc // NPC
        e_l = esrc % NPC
        e_k = np.searchsorted(cc_bounds, e_l, side="right") - 1
        r0s = np.asarray(cc_bounds[:-1])[e_k]
        szs = (np.asarray(cc_bounds[1:]) - np.asarray(cc_bounds[:-1]))[e_k]
        esrc2 = NCORE * r0s + e_c * szs + (e_l - r0s)
        per_core.append({
            "x": x_pad, "esrc2": esrc2.astype(np.int32),
            "xt_own": np.ascontiguousarray(xt[:, c * NPC:c * NPC + NPAD]),
            "esrc": esrc, "dstl": dstl, "dstlrow": dstlrow,
            "w1cat": w1cat, "wad1": w_ad1, "w2cat": w2cat,
            "b1": np.asarray(b1, np.float32).reshape(1, HC),
            "b2": np.asarray(b2, np.float32).reshape(1, C),
        })
    return per_core, cap


# ---------- cached PJRT runner (axon path), self-contained ----------

def make_runner(nc, n_cores):
    import jax
    from jax.sharding import Mesh, PartitionSpec, NamedSharding
    from jax.experimental.shard_map import shard_map
    from concourse import mybir as mb
    from concourse.bass2jax import (_bass_exec_p, install_neuronx_cc_hook,
                                    partition_id_tensor)

    install_neuronx_cc_hook()
    in_names, out_names, out_avals = [], [], []
    pid_name = nc.partition_id_tensor.name if nc.partition_id_tensor else None
    for alloc in nc.m.functions[0].allocations:
        if not isinstance(alloc, mb.MemoryLocationSet):
            continue
        name = alloc.memorylocations[0].name
        if alloc.kind == "ExternalInput":
            if name != pid_name:
                in_names.append(name)
        elif alloc.kind == "ExternalOutput":
            out_names.append(name)
            out_avals.append(jax.core.ShapedArray(
                tuple(alloc.tensor_shape), mb.dt.np(alloc.dtype)))
    n_params = len(in_names)
    all_in = in_names + out_names + ([pid_name] if pid_name else [])
    donate = tuple(range(n_params, n_params + len(out_names)))

    def _body(*args):
        args = list(args)
        if pid_name is not None:
            args.append(partition_id_tensor())
        return tuple(_bass_exec_p.bind(
            *args, out_avals=tuple(out_avals), in_names=tuple(all_in),
            out_names=tuple(out_names), lowering_input_output_aliases=(),
            sim_require_finite=True, sim_require_nnan=True, nc=nc))

    devices = jax.devices()[:n_cores]
    mesh = Mesh(np.asarray(devices), ("core",))
    jit_fn = jax.jit(
        shard_map(_body, mesh=mesh,
                  in_specs=(PartitionSpec("core"),) * (n_params + len(out_names)),
                  out_specs=(PartitionSpec("core"),) * len(out_names)),
        donate_argnums=donate, keep_unused=True)
    sharding = NamedSharding(mesh, PartitionSpec("core"))
    dev_cache = {}

    def run(in_maps, time_only=False):
        import time as _t
        ins = []
        for name in in_names:
            key = (name,) + tuple(id(m[name]) for m in in_maps)
            if key not in dev_cache:
                cat = np.concatenate([np.asarray(m[name]) for m in in_maps],
                                     axis=0)
                dev_cache[key] = jax.device_put(cat, sharding)
            ins.append(dev_cache[key])
        zeros = [jax.device_put(
            np.zeros((av.shape[0] * n_cores,) + av.shape[1:], av.dtype),
            sharding) for av in out_avals]
        jax.block_until_ready(zeros)
        t0 = _t.time()
        r = jit_fn(*ins, *zeros)
        jax.block_until_ready(r)
        dt = _t.time() - t0
        if time_only:
            return dt
        outs = []
        for ci in range(n_cores):
            d = {}
            for i, nm in enumerate(out_names):
                full = np.asarray(r[i])
                per = full.shape[0] // n_cores
                d[nm] = full[ci * per:(ci + 1) * per]
            outs.append(d)
        return outs, dt

    return run


def get_runner(cap, repeat=1):
    key = (cap, repeat)
    if key not in _cache:
        nc = build_program(cap, repeat)
        _cache[key] = make_runner(nc, NCORE)
    return _cache[key]


def kernel(**inputs):
    per_core, cap = preprocess(**inputs)
    run = get_runner(cap, 1)
    res, _ = run(per_core)
    out = np.concatenate([res[c]["out_own"][:NPC] for c in range(NCORE)],
                         axis=0)
    return np.ascontiguousarray(out, dtype=np.float32)

